# revision 3
# baseline (speedup 1.0000x reference)
"""AGCRN cell kernel for 8 Trainium2 NeuronCores.

Strategy: data-parallel over batch (B=32 -> 4 per core). Each core
redundantly builds S = exp(relu(E E^T)) (symmetric, so it serves directly
as the matmul stationary operand without any transpose) plus row sums d;
the adaptive-adjacency normalization 1/d is folded into PSUM evacuations
as a per-partition scale. The Chebyshev chain runs node-major; the
x_g @ W contraction transposes 96-column chunks on the PE (zero-padded W
rows absorb pad garbage, a ones-column provides the bias for free).
"""

import os
import sys

import numpy as np
import ml_dtypes

for _p in ("/opt/trn_rl_repo", "/root/.axon_site/_ro/trn_rl_repo"):
    if os.path.isdir(_p) and _p not in sys.path:
        sys.path.append(_p)

import concourse.bass as bass
import concourse.tile as tile
from concourse import bacc, mybir
from concourse.bass_utils import run_bass_kernel_spmd
from concourse.masks import make_identity

F32 = mybir.dt.float32
BF16 = mybir.dt.bfloat16
AF = mybir.ActivationFunctionType
ALU = mybir.AluOpType

P = 128          # partitions
N = 2048         # nodes
NT = N // P      # node tiles = 16
NB = 4           # batches per core
CH = 66          # dim_in + hidden
CPB = 96         # padded channel slot per batch (66 real + 1 ones + 29 pad)
HID = 64
OC_G = 128       # gate output channels (2*hidden)
NCORES = 8
RT_GROUP = 8     # row-tiles per transpose/matmul group


def _dv(ap, nb=NB, w=CPB):
    """View a [P, nb*w] slice as [P, nb, w]."""
    return ap.rearrange("p (b c) -> p b c", b=nb)


def build_nc():
    nc = bacc.Bacc(
        "TRN2",
        target_bir_lowering=False,
        debug=False,
        enable_asserts=False,
        num_devices=NCORES,
    )
    x_d = nc.dram_tensor("x", [NB, N, 2], F32, kind="ExternalInput").ap()
    st_d = nc.dram_tensor("state", [NB, N, HID], F32, kind="ExternalInput").ap()
    et_d = nc.dram_tensor("et", [10, N], F32, kind="ExternalInput").ap()
    wg_d = nc.dram_tensor("wg", [3, CPB, OC_G], BF16, kind="ExternalInput").ap()
    wu_d = nc.dram_tensor("wu", [3, CPB, HID], BF16, kind="ExternalInput").ap()
    out_d = nc.dram_tensor("out", [NB, N, HID], F32, kind="ExternalOutput").ap()

    with tile.TileContext(nc) as tc:
        _build(tc, x_d, st_d, et_d, wg_d, wu_d, out_d)
    nc.compile()
    return nc


def _build(tc, x_d, st_d, et_d, wg_d, wu_d, out_d):
    nc = tc.nc
    from contextlib import ExitStack

    with ExitStack() as ctx:
        const = ctx.enter_context(tc.tile_pool(name="const", bufs=1))
        persist = ctx.enter_context(tc.tile_pool(name="persist", bufs=1))

        ident = const.tile([P, P], BF16)
        make_identity(nc, ident)

        et_sb = const.tile([10, N], F32)
        nc.sync.dma_start(et_sb[:], et_d[:])
        wg_sb = const.tile([CPB, 3, OC_G], BF16)
        wu_sb = const.tile([CPB, 3, HID], BF16)
        for k in range(3):
            nc.sync.dma_start(wg_sb[:, k, :], wg_d[k])
            nc.sync.dma_start(wu_sb[:, k, :], wu_d[k])

        S_sb = persist.tile([P, NT, N], BF16)       # S row-tiles
        x0_sb = persist.tile([P, NT, NB * CPB], BF16)
        u1_sb = persist.tile([P, NT, NB * CPB], BF16)
        u2_sb = persist.tile([P, NT, NB * CPB], BF16)
        stt_sb = persist.tile([P, NT, NB, HID], BF16)   # state copy for epilogue
        zr_sb = persist.tile([P, NT, NB, OC_G], BF16)   # sigmoid(gate)
        d_sb = persist.tile([P, NT, 4], F32)
        dtot = persist.tile([P, NT], F32)
        rinv = persist.tile([P, NT], F32)
        rinv2 = persist.tile([P, NT], F32)

        nc.vector.memset(x0_sb[:], 0.0)
        nc.vector.memset(u1_sb[:], 0.0)
        nc.vector.memset(u2_sb[:], 0.0)
        for b in range(NB):
            # ones column feeding the bias row of W chunk 2
            nc.vector.memset(u2_sb[:, :, b * CPB + CH : b * CPB + CH + 1], 1.0)

        # ---- input load + f32->bf16 convert ----
        inp_pool = ctx.enter_context(tc.tile_pool(name="inp", bufs=3))
        for nt in range(NT):
            stf = inp_pool.tile([P, NB, HID], F32, tag="stf")
            xf = inp_pool.tile([P, NB, 2], F32, tag="xf")
            for b in range(NB):
                nc.sync.dma_start(stf[:, b, :], st_d[b, nt * P : (nt + 1) * P, :])
                nc.sync.dma_start(xf[:, b, :], x_d[b, nt * P : (nt + 1) * P, :])
            x0v = _dv(x0_sb[:, nt, :])
            nc.vector.tensor_copy(x0v[:, :, 2 : 2 + HID], stf[:])
            nc.vector.tensor_copy(x0v[:, :, 0:2], xf[:])
            nc.vector.tensor_copy(stt_sb[:, nt], stf[:])

        # ---- S = exp(relu(E E^T)) with row sums ----
        with tc.tile_pool(name="lpsum", bufs=3, space="PSUM") as lpsum:
            for mt in range(NT):
                for q in range(4):
                    lp = lpsum.tile([P, 512], F32)
                    nc.tensor.matmul(
                        lp[:],
                        lhsT=et_sb[:, mt * P : (mt + 1) * P],
                        rhs=et_sb[:, q * 512 : (q + 1) * 512],
                        start=True,
                        stop=True,
                    )
                    s_sl = S_sb[:, mt, q * 512 : (q + 1) * 512]
                    nc.scalar.activation(s_sl, lp[:], AF.Exp)
                    # exp(relu(x)) == max(exp(x), 1); accumulate row sums
                    nc.vector.tensor_scalar(
                        s_sl,
                        s_sl,
                        1.0,
                        None,
                        op0=ALU.max,
                        op1=ALU.add,
                        accum_out=d_sb[:, mt, q : q + 1],
                    )
                nc.vector.tensor_reduce(
                    dtot[:, mt : mt + 1],
                    d_sb[:, mt, :],
                    axis=mybir.AxisListType.X,
                    op=ALU.add,
                )
                nc.vector.reciprocal(rinv[:, mt : mt + 1], dtot[:, mt : mt + 1])
                nc.vector.tensor_scalar_mul(
                    rinv2[:, mt : mt + 1], rinv[:, mt : mt + 1], 2.0
                )

        cpsum = ctx.enter_context(tc.tile_pool(name="cpsum", bufs=2, space="PSUM"))
        tpsum = ctx.enter_context(tc.tile_pool(name="tpsum", bufs=3, space="PSUM"))
        zpsum = ctx.enter_context(tc.tile_pool(name="zpsum", bufs=2, space="PSUM"))
        xgt_pool = ctx.enter_context(tc.tile_pool(name="xgt", bufs=3 * RT_GROUP))
        epi_pool = ctx.enter_context(tc.tile_pool(name="epi", bufs=4))

        def apply_S(src, dst, second):
            """dst = (S @ src) / d   (or 2*(S @ src)/d - x0 when second)."""
            for mt in range(NT):
                cp = cpsum.tile([P, NB * CH], F32)
                for kt in range(NT):
                    nc.tensor.matmul(
                        cp[:],
                        lhsT=S_sb[:, kt, mt * P : (mt + 1) * P],
                        rhs=_dv(src[:, kt, :])[:, :, 0:CH],
                        start=(kt == 0),
                        stop=(kt == NT - 1),
                    )
                dstv = _dv(dst[:, mt, :])[:, :, 0:CH]
                if not second:
                    nc.scalar.activation(
                        dstv, cp[:], AF.Copy, scale=rinv[:, mt : mt + 1]
                    )
                else:
                    nc.vector.scalar_tensor_tensor(
                        out=dstv,
                        in0=cp[:],
                        scalar=rinv2[:, mt : mt + 1],
                        in1=_dv(x0_sb[:, mt, :])[:, :, 0:CH],
                        op0=ALU.mult,
                        op1=ALU.subtract,
                    )

        def gconv_tail(gate):
            """Transpose x_g chunks + W matmul + nonlinearity (+ epilogue)."""
            w_sb = wg_sb if gate else wu_sb
            oc = OC_G if gate else HID
            rts = [(nt, b) for nt in range(NT) for b in range(NB)]
            for g0 in range(0, len(rts), RT_GROUP):
                group = rts[g0 : g0 + RT_GROUP]
                xgts = {}
                for nt, b in group:
                    for k, src in enumerate((x0_sb, u1_sb, u2_sb)):
                        tp = tpsum.tile([CPB, P], BF16)
                        nc.tensor.transpose(
                            tp[:],
                            src[:, nt, b * CPB : (b + 1) * CPB],
                            ident[:],
                        )
                        xgt = xgt_pool.tile([CPB, P], BF16)
                        nc.vector.tensor_copy(xgt[:], tp[:])
                        xgts[(nt, b, k)] = xgt
                for nt, b in group:
                    zp = zpsum.tile([P, oc], F32, tag="zp")
                    for k in range(3):
                        nc.tensor.matmul(
                            zp[:],
                            lhsT=xgts[(nt, b, k)][:],
                            rhs=w_sb[:, k, :],
                            start=(k == 0),
                            stop=(k == 2),
                        )
                    if gate:
                        nc.scalar.activation(zr_sb[:, nt, b, :], zp[:], AF.Sigmoid)
                        # candidate: state-cols of x0 *= z  (in place)
                        x0c = x0_sb[:, nt, b * CPB + 2 : b * CPB + 2 + HID]
                        nc.vector.tensor_mul(x0c, x0c, zr_sb[:, nt, b, 0:HID])
                    else:
                        hc = epi_pool.tile([P, HID], BF16, tag="hc")
                        nc.scalar.activation(hc[:], zp[:], AF.Tanh)
                        r = zr_sb[:, nt, b, HID:OC_G]
                        t1 = epi_pool.tile([P, HID], BF16, tag="t1")
                        nc.vector.tensor_sub(t1[:], stt_sb[:, nt, b, :], hc[:])
                        hf = epi_pool.tile([P, HID], F32, tag="hf")
                        # h = hc + r*(state - hc)
                        nc.vector.scalar_tensor_tensor(
                            out=hf[:],
                            in0=t1[:],
                            scalar=1.0,
                            in1=r,
                            op0=ALU.mult,
                            op1=ALU.mult,
                        )
                        nc.vector.tensor_add(hf[:], hf[:], hc[:])
                        nc.sync.dma_start(
                            out_d[b, nt * P : (nt + 1) * P, :], hf[:]
                        )

        # gconv 1 (gate)
        apply_S(x0_sb, u1_sb, second=False)
        apply_S(u1_sb, u2_sb, second=True)
        gconv_tail(gate=True)
        # gconv 2 (update) -- x0_sb now holds the candidate input
        apply_S(x0_sb, u1_sb, second=False)
        apply_S(u1_sb, u2_sb, second=True)
        gconv_tail(gate=False)


_NC = None


def _get_nc():
    global _NC
    if _NC is None:
        _NC = build_nc()
    return _NC


def _prep_in_maps(x, state, node_embeddings, W_gate, b_gate, W_update, b_update):
    bf = ml_dtypes.bfloat16
    x = np.asarray(x, dtype=np.float32)
    state = np.asarray(state, dtype=np.float32)
    E = np.asarray(node_embeddings, dtype=np.float32)
    W_gate = np.asarray(W_gate, dtype=np.float32)
    b_gate = np.asarray(b_gate, dtype=np.float32)
    W_update = np.asarray(W_update, dtype=np.float32)
    b_update = np.asarray(b_update, dtype=np.float32)

    et = np.ascontiguousarray(E.T)  # [10, N]
    wg = np.zeros((3, CPB, OC_G), np.float32)
    wu = np.zeros((3, CPB, HID), np.float32)
    for k in range(3):
        wg[k, :CH] = W_gate[CH * k : CH * (k + 1)]
        wu[k, :CH] = W_update[CH * k : CH * (k + 1)]
    wg[2, CH] = b_gate
    wu[2, CH] = b_update
    wg = wg.astype(bf)
    wu = wu.astype(bf)

    in_maps = []
    for r in range(NCORES):
        in_maps.append(
            {
                "x": np.ascontiguousarray(x[NB * r : NB * (r + 1)]),
                "state": np.ascontiguousarray(state[NB * r : NB * (r + 1)]),
                "et": et,
                "wg": wg,
                "wu": wu,
            }
        )
    return in_maps


def run(trace=False, **inputs):
    nc = _get_nc()
    in_maps = _prep_in_maps(**inputs)
    res = run_bass_kernel_spmd(
        nc, in_maps, core_ids=list(range(NCORES)), trace=trace
    )
    out = np.concatenate([res.results[r]["out"] for r in range(NCORES)], axis=0)
    return out, res


def kernel(**inputs) -> np.ndarray:
    out, _ = run(trace=False, **inputs)
    return out


# revision 7
# speedup vs baseline: 1.0697x; 1.0697x over previous
"""AGCRN cell kernel for 8 Trainium2 NeuronCores.

Strategy: data-parallel over batch (B=32 -> 4 per core). Each core
redundantly builds S = exp(relu(E E^T)) (symmetric, so it serves directly
as the matmul stationary operand without any transpose) plus row sums d;
the adaptive-adjacency normalization 1/d is folded into PSUM evacuations
as a per-partition scale. The Chebyshev chain runs node-major; the
x_g @ W contraction transposes 96-column chunks on the PE (zero-padded W
rows absorb pad garbage, a ones-column provides the bias for free).
"""

import os
import sys

import numpy as np
import ml_dtypes

for _p in ("/opt/trn_rl_repo", "/root/.axon_site/_ro/trn_rl_repo"):
    if os.path.isdir(_p) and _p not in sys.path:
        sys.path.append(_p)

import concourse.bass as bass
import concourse.tile as tile
from concourse import bacc, mybir
from concourse.bass_utils import run_bass_kernel_spmd
from concourse.masks import make_identity

F32 = mybir.dt.float32
BF16 = mybir.dt.bfloat16
AF = mybir.ActivationFunctionType
ALU = mybir.AluOpType

P = 128          # partitions
N = 2048         # nodes
NT = N // P      # node tiles = 16
NB = 4           # batches per core
CH = 66          # dim_in + hidden
CPB = 96         # padded channel slot per batch (66 real + 1 ones + 29 pad)
HID = 64
OC_G = 128       # gate output channels (2*hidden)
NCORES = 8
RT_GROUP = 16    # row-tiles per transpose/matmul group


def _dv(ap, nb=NB, w=CPB):
    """View a [P, nb*w] slice as [P, nb, w]."""
    return ap.rearrange("p (b c) -> p b c", b=nb)


def build_nc():
    nc = bacc.Bacc(
        "TRN2",
        target_bir_lowering=False,
        debug=False,
        enable_asserts=False,
        num_devices=NCORES,
    )
    x_d = nc.dram_tensor("x", [NB, N, 2], F32, kind="ExternalInput").ap()
    st_d = nc.dram_tensor("state", [NB, N, HID], F32, kind="ExternalInput").ap()
    et_d = nc.dram_tensor("et", [10, N], F32, kind="ExternalInput").ap()
    wg_d = nc.dram_tensor("wg", [3, CPB, OC_G], BF16, kind="ExternalInput").ap()
    wu_d = nc.dram_tensor("wu", [3, CPB, HID], BF16, kind="ExternalInput").ap()
    out_d = nc.dram_tensor("out", [NB, N, HID], F32, kind="ExternalOutput").ap()

    with tile.TileContext(nc) as tc:
        _build(tc, x_d, st_d, et_d, wg_d, wu_d, out_d)
    nc.compile()
    return nc


def _build(tc, x_d, st_d, et_d, wg_d, wu_d, out_d):
    nc = tc.nc
    from contextlib import ExitStack

    with ExitStack() as ctx:
        const = ctx.enter_context(tc.tile_pool(name="const", bufs=1))
        persist = ctx.enter_context(tc.tile_pool(name="persist", bufs=1))

        ident = const.tile([P, P], BF16)
        make_identity(nc, ident)

        et_sb = const.tile([10, N], F32)
        nc.sync.dma_start(et_sb[:], et_d[:])
        et_r = const.tile([10, N], mybir.dt.float32r)
        nc.vector.tensor_copy(et_r[:], et_sb[:])
        wg_sb = const.tile([CPB, 3, OC_G], BF16)
        wu_sb = const.tile([CPB, 3, HID], BF16)
        for k in range(3):
            nc.sync.dma_start(wg_sb[:, k, :], wg_d[k])
            nc.sync.dma_start(wu_sb[:, k, :], wu_d[k])

        S_sb = persist.tile([P, NT, N], BF16)       # S row-tiles
        x0_sb = persist.tile([P, NT, NB * CPB], BF16)
        u1_sb = persist.tile([P, NT, NB * CPB], BF16)
        u2_sb = persist.tile([P, NT, NB * CPB], BF16)
        stt_sb = persist.tile([P, NT, NB, HID], BF16)   # state copy for epilogue
        zr_sb = persist.tile([P, NT, NB, OC_G], BF16)   # sigmoid(gate)
        d_sb = persist.tile([P, NT, 4], F32)
        dtot = persist.tile([P, NT], F32)
        rinv = persist.tile([P, NT], F32)
        rinv2 = persist.tile([P, NT], F32)

        nc.vector.memset(x0_sb[:], 0.0)
        nc.vector.memset(u1_sb[:], 0.0)
        nc.vector.memset(u2_sb[:], 0.0)
        for b in range(NB):
            # ones column feeding the bias row of W chunk 2
            nc.vector.memset(u2_sb[:, :, b * CPB + CH : b * CPB + CH + 1], 1.0)

        # ---- input load + f32->bf16 convert ----
        inp_pool = ctx.enter_context(tc.tile_pool(name="inp", bufs=3))
        for nt in range(NT):
            stf = inp_pool.tile([P, NB, HID], F32, tag="stf")
            xf = inp_pool.tile([P, NB, 2], F32, tag="xf")
            for b in range(NB):
                nc.sync.dma_start(stf[:, b, :], st_d[b, nt * P : (nt + 1) * P, :])
                nc.sync.dma_start(xf[:, b, :], x_d[b, nt * P : (nt + 1) * P, :])
            x0v = _dv(x0_sb[:, nt, :])
            nc.vector.tensor_copy(x0v[:, :, 2 : 2 + HID], stf[:])
            nc.vector.tensor_copy(x0v[:, :, 0:2], xf[:])
            nc.vector.tensor_copy(stt_sb[:, nt], stf[:])

        # ---- S = exp(relu(E E^T)) with row sums ----
        with tc.tile_pool(name="lpsum", bufs=3, space="PSUM") as lpsum:
            for mt in range(NT):
                for q in range(4):
                    lp = lpsum.tile([P, 512], F32)
                    nc.tensor.matmul(
                        lp[:],
                        lhsT=et_r[:, mt * P : (mt + 1) * P],
                        rhs=et_r[:, q * 512 : (q + 1) * 512],
                        start=True,
                        stop=True,
                    )
                    s_sl = S_sb[:, mt, q * 512 : (q + 1) * 512]
                    nc.scalar.activation(s_sl, lp[:], AF.Exp)
                    # exp(relu(x)) == max(exp(x), 1); accumulate row sums
                    nc.vector.tensor_scalar(
                        s_sl,
                        s_sl,
                        1.0,
                        None,
                        op0=ALU.max,
                        op1=ALU.add,
                        accum_out=d_sb[:, mt, q : q + 1],
                    )
                nc.vector.tensor_reduce(
                    dtot[:, mt : mt + 1],
                    d_sb[:, mt, :],
                    axis=mybir.AxisListType.X,
                    op=ALU.add,
                )
                nc.vector.reciprocal(rinv[:, mt : mt + 1], dtot[:, mt : mt + 1])
                nc.vector.tensor_scalar_mul(
                    rinv2[:, mt : mt + 1], rinv[:, mt : mt + 1], 2.0
                )

        cpsum = ctx.enter_context(tc.tile_pool(name="cpsum", bufs=2, space="PSUM"))
        tpsum = ctx.enter_context(tc.tile_pool(name="tpsum", bufs=3, space="PSUM"))
        zpsum = ctx.enter_context(tc.tile_pool(name="zpsum", bufs=2, space="PSUM"))
        xgt_pool = ctx.enter_context(tc.tile_pool(name="xgt", bufs=3 * RT_GROUP))
        epi_pool = ctx.enter_context(tc.tile_pool(name="epi", bufs=4))

        def apply_S(src, dst, second):
            """dst = (S @ src) / d   (or 2*(S @ src)/d - x0 when second)."""
            for mt in range(NT):
                cp = cpsum.tile([P, NB * CH], F32)
                for kt in range(NT):
                    nc.tensor.matmul(
                        cp[:],
                        lhsT=S_sb[:, kt, mt * P : (mt + 1) * P],
                        rhs=_dv(src[:, kt, :])[:, :, 0:CH],
                        start=(kt == 0),
                        stop=(kt == NT - 1),
                    )
                dstv = _dv(dst[:, mt, :])[:, :, 0:CH]
                if not second:
                    nc.scalar.activation(
                        dstv, cp[:], AF.Copy, scale=rinv[:, mt : mt + 1]
                    )
                else:
                    nc.vector.scalar_tensor_tensor(
                        out=dstv,
                        in0=cp[:],
                        scalar=rinv2[:, mt : mt + 1],
                        in1=_dv(x0_sb[:, mt, :])[:, :, 0:CH],
                        op0=ALU.mult,
                        op1=ALU.subtract,
                    )

        def gconv_tail(gate):
            """Transpose x_g chunks + W matmul + nonlinearity (+ epilogue)."""
            w_sb = wg_sb if gate else wu_sb
            oc = OC_G if gate else HID
            rts = [(nt, b) for nt in range(NT) for b in range(NB)]
            for g0 in range(0, len(rts), RT_GROUP):
                group = rts[g0 : g0 + RT_GROUP]
                xgts = {}
                for nt, b in group:
                    for k, src in enumerate((x0_sb, u1_sb, u2_sb)):
                        tp = tpsum.tile([CPB, P], BF16)
                        nc.tensor.transpose(
                            tp[:],
                            src[:, nt, b * CPB : (b + 1) * CPB],
                            ident[:],
                        )
                        xgt = xgt_pool.tile([CPB, P], BF16)
                        nc.vector.tensor_copy(xgt[:], tp[:])
                        xgts[(nt, b, k)] = xgt
                for nt, b in group:
                    zp = zpsum.tile([P, oc], F32, tag="zp")
                    for k in range(3):
                        nc.tensor.matmul(
                            zp[:],
                            lhsT=xgts[(nt, b, k)][:],
                            rhs=w_sb[:, k, :],
                            start=(k == 0),
                            stop=(k == 2),
                        )
                    if gate:
                        nc.scalar.activation(zr_sb[:, nt, b, :], zp[:], AF.Sigmoid)
                        # candidate: state-cols of x0 *= z  (in place)
                        x0c = x0_sb[:, nt, b * CPB + 2 : b * CPB + 2 + HID]
                        nc.vector.tensor_mul(x0c, x0c, zr_sb[:, nt, b, 0:HID])
                    else:
                        hc = epi_pool.tile([P, HID], BF16, tag="hc")
                        nc.scalar.activation(hc[:], zp[:], AF.Tanh)
                        r = zr_sb[:, nt, b, HID:OC_G]
                        t1 = epi_pool.tile([P, HID], BF16, tag="t1")
                        nc.vector.tensor_sub(t1[:], stt_sb[:, nt, b, :], hc[:])
                        hf = epi_pool.tile([P, HID], F32, tag="hf")
                        # h = hc + r*(state - hc)
                        nc.vector.scalar_tensor_tensor(
                            out=hf[:],
                            in0=t1[:],
                            scalar=1.0,
                            in1=r,
                            op0=ALU.mult,
                            op1=ALU.mult,
                        )
                        nc.vector.tensor_add(hf[:], hf[:], hc[:])
                        nc.sync.dma_start(
                            out_d[b, nt * P : (nt + 1) * P, :], hf[:]
                        )

        # gconv 1 (gate)
        apply_S(x0_sb, u1_sb, second=False)
        apply_S(u1_sb, u2_sb, second=True)
        gconv_tail(gate=True)
        # gconv 2 (update) -- x0_sb now holds the candidate input
        apply_S(x0_sb, u1_sb, second=False)
        apply_S(u1_sb, u2_sb, second=True)
        gconv_tail(gate=False)


_NC = None


def _get_nc():
    global _NC
    if _NC is None:
        _NC = build_nc()
    return _NC


def _prep_in_maps(x, state, node_embeddings, W_gate, b_gate, W_update, b_update):
    bf = ml_dtypes.bfloat16
    x = np.asarray(x, dtype=np.float32)
    state = np.asarray(state, dtype=np.float32)
    E = np.asarray(node_embeddings, dtype=np.float32)
    W_gate = np.asarray(W_gate, dtype=np.float32)
    b_gate = np.asarray(b_gate, dtype=np.float32)
    W_update = np.asarray(W_update, dtype=np.float32)
    b_update = np.asarray(b_update, dtype=np.float32)

    et = np.ascontiguousarray(E.T)  # [10, N]
    wg = np.zeros((3, CPB, OC_G), np.float32)
    wu = np.zeros((3, CPB, HID), np.float32)
    for k in range(3):
        wg[k, :CH] = W_gate[CH * k : CH * (k + 1)]
        wu[k, :CH] = W_update[CH * k : CH * (k + 1)]
    wg[2, CH] = b_gate
    wu[2, CH] = b_update
    wg = wg.astype(bf)
    wu = wu.astype(bf)

    in_maps = []
    for r in range(NCORES):
        in_maps.append(
            {
                "x": np.ascontiguousarray(x[NB * r : NB * (r + 1)]),
                "state": np.ascontiguousarray(state[NB * r : NB * (r + 1)]),
                "et": et,
                "wg": wg,
                "wu": wu,
            }
        )
    return in_maps


def run(trace=False, **inputs):
    nc = _get_nc()
    in_maps = _prep_in_maps(**inputs)
    res = run_bass_kernel_spmd(
        nc, in_maps, core_ids=list(range(NCORES)), trace=trace
    )
    out = np.concatenate([res.results[r]["out"] for r in range(NCORES)], axis=0)
    return out, res


def kernel(**inputs) -> np.ndarray:
    out, _ = run(trace=False, **inputs)
    return out
